# revision 29
# baseline (speedup 1.0000x reference)
"""Trainium2 Bass kernel for nn_FAA_51367808860389 (FAN-attention w/ dynamic-graph bias).

Strategy: data-parallel over batch B=32 across 8 cores (4 batches/core).
Everything computed in transposed orientation energyT[k,q]:
  - FAN q/k projections via ONE matmul per batch (stationary [41,80] with
    pa duplicated so cos- and sin-source rows land on distinct partitions),
    compact 80-row layout, per-head channel rows contiguous -> no masking.
  - energyT[k,q] per (head, k-block): K=5 matmul from row slices.
  - bias via diagonal-matmul: eT block += dg-block^T @ diag(w-block); dg
    streamed from HBM in fp8e4m3 (half DMA), one DMA per (batch, head).
  - exp on Act engine writes att directly in fp8; out-matmul uses fp8
    DoubleRow (pairs of k-blocks contracted per pass) with an appended
    ones column for the softmax denominators.
  - normalize + head-concat + out-projection fused via prepacked matrices.
Output produced transposed [40, 512] per batch; host transposes back.
"""
import numpy as np

B, N, E, H, D = 32, 512, 40, 8, 5
NCORES = 8
B_LOC = B // NCORES
SCALE = 1.0 / float(np.float32(E) ** 0.5)

# qT/kT row layout (identical for both): cos ch0-9 at rows 0-9 | pad 10-31 |
# sin ch10-19 at rows 32-41 | pad 42-63 | g ch20-39 at rows 64-83.
# 32-aligned groups (Act/DVE/matmul partition bases must be multiples of 32).
QKR = 84
R_CH = {h: (5 * h if h < 2 else (22 + 5 * h if h < 4 else 44 + 5 * h))
        for h in range(H)}

_PROG_CACHE = {}


def _build_program():
    if "nc" in _PROG_CACHE:
        return _PROG_CACHE["nc"]
    import concourse.bass as bass
    import concourse.tile as tile
    from concourse import bacc, mybir

    F32 = mybir.dt.float32
    BF16 = mybir.dt.bfloat16
    FP8 = mybir.dt.float8e4
    AF = mybir.ActivationFunctionType
    OP = mybir.AluOpType
    DR = mybir.MatmulPerfMode.DoubleRow

    nc = bacc.Bacc(None)
    dp = nc.declare_dram_parameter
    xta_d = dp("xta", [B_LOC, 41, N], BF16, isOutput=False)
    dg1_d = dp("dg1", [B_LOC, 4, N, N], FP8, isOutput=False)
    dg2_d = dp("dg2", [B_LOC, 4, N, N], FP8, isOutput=False)
    wq_d = dp("wq", [41, QKR], BF16, isOutput=False)
    wk_d = dp("wk", [41, QKR], BF16, isOutput=False)
    wv_d = dp("wv", [41, 30], BF16, isOutput=False)     # v fan weights
    dgw80_d = dp("dgw80", [QKR, 2], BF16, isOutput=False)
    masks_d = dp("masks", [QKR, H], F32, isOutput=False)
    dgb_d = dp("dgb", [1, 2], BF16, isOutput=False)
    sel_lo_d = dp("sel_lo", [128, 8], BF16, isOutput=False)
    sel_hi_d = dp("sel_hi", [128, 8], BF16, isOutput=False)
    e5_lo_d = dp("e5_lo", [8, 128], BF16, isOutput=False)
    e5_hi_d = dp("e5_hi", [8, 128], BF16, isOutput=False)
    p_lo_d = dp("p_lo", [128, E], BF16, isOutput=False)
    p_hi_d = dp("p_hi", [128, E], BF16, isOutput=False)
    projb_d = dp("projb", [E, 1], F32, isOutput=False)
    i128_d = dp("i128", [128, 128], BF16, isOutput=False)
    out_d = dp("outT", [B_LOC, E, N], F32, isOutput=True)

    lp = nc.allow_low_precision(reason="bf16/fp8 datapath validated vs reference")
    lp.__enter__()
    with tile.TileContext(nc) as tc:
        with (
            tc.tile_pool(name="const", bufs=1) as cp,
            tc.tile_pool(name="work", bufs=2) as wp,
            tc.tile_pool(name="persist", bufs=B_LOC) as pp,
            tc.tile_pool(name="wcolp", bufs=8 * B_LOC) as wcp,
            tc.tile_pool(name="dgp", bufs=B_LOC) as dgpool,
            tc.tile_pool(name="kmp", bufs=2) as kmp,
            tc.tile_pool(name="attp", bufs=6) as attp,
            tc.tile_pool(name="psE", bufs=2, space=bass.MemorySpace.PSUM) as psE,
            tc.tile_pool(name="psO", bufs=1, space=bass.MemorySpace.PSUM) as psO,
            tc.tile_pool(name="psS", bufs=1, space=bass.MemorySpace.PSUM) as psS,
        ):
            # ---- constants to SBUF ----
            def cload(dram, shape, tag, dt=F32):
                t = cp.tile(shape, dt, tag=tag)
                nc.sync.dma_start(t[:], dram[:])
                return t

            wq = cload(wq_d, [41, QKR], "wq", BF16)
            wk = cload(wk_d, [41, QKR], "wk", BF16)
            wv = cload(wv_d, [41, 30], "wv", BF16)
            dgw80 = cload(dgw80_d, [QKR, 2], "dgw80", BF16)
            masks = cload(masks_d, [QKR, H], "masks")
            dgb = cload(dgb_d, [1, 2], "dgb", BF16)
            sel_lo = cload(sel_lo_d, [128, 8], "sel_lo", BF16)
            sel_hi = cload(sel_hi_d, [128, 8], "sel_hi", BF16)
            e5_lo = cload(e5_lo_d, [8, 128], "e5_lo", BF16)
            e5_hi = cload(e5_hi_d, [8, 128], "e5_hi", BF16)
            p_lo = cload(p_lo_d, [128, E], "p_lo", BF16)
            p_hi = cload(p_hi_d, [128, E], "p_hi", BF16)
            projb = cload(projb_d, [E, 1], "projb")
            i128 = cload(i128_d, [128, 128], "i128", BF16)
            ones_row = cp.tile([1, 128], BF16, tag="ones_row")
            nc.vector.memset(ones_row[:], 1.0)
            # PE warm-up: HAM throttles the PE to half clock until it sees
            # ~3us of continuous matmul activity. Burn a few back-to-back
            # dummy matmuls at the start (and between phases) so the whole
            # main loop runs at full clock.
            warm_src = cp.tile([128, N], BF16, tag="warm_src")
            nc.vector.memset(warm_src[:], 0.0)

            warm_w = cp.tile([128, 1], BF16, tag="warm_w")
            nc.vector.memset(warm_w[:], 0.0)

            def pe_warm(k):
                # real matmuls into a dedicated 1-row psum tile: HAM only
                # un-throttles the PE on sustained *matmul* activity. The
                # WAW chain on one tile stays within the PE queue, so these
                # fill idle windows without cross-engine dependencies.
                for _ in range(k):
                    pw = psS.tile([1, N], F32, tag="warm", bufs=1)
                    nc.tensor.matmul(pw[:], warm_w[:], warm_src[:],
                                     start=True, stop=True)

            pe_warm(8)

            # ---- xta loads + dg prefetch (all upfront) ----
            xta = []
            for b in range(B_LOC):
                xt = pp.tile([41, N], BF16, tag="xta")
                nc.sync.dma_start(xt[:], xta_d[b][:])
                xta.append(xt)
            # dg tiles: [128, 2048] fp8, free = r*512 + n (r = q-row-block)
            dgt = [[None] * H for _ in range(B_LOC)]
            for b in range(B_LOC):
                for h in range(H):
                    src = dg1_d if h < 4 else dg2_d
                    t = dgpool.tile([128, 4 * N], FP8, tag=f"dg{h}")
                    nc.sync.dma_start(
                        t[:].rearrange("p (r n) -> p r n", r=4),
                        src[b, h % 4].rearrange("(r p) n -> p r n", p=128))
                    dgt[b][h] = t

            # ---- phase F: FAN q/k for all local batches ----
            def fan_qk(b, w, tag):
                ps = psS.tile([QKR, N], F32, tag="fan")
                nc.tensor.matmul(ps[:], w[:], xta[b][:], start=True, stop=True)
                qk = pp.tile([QKR, N], BF16, tag=tag)
                s2 = wp.tile([64, N], F32, tag="s2")
                s4 = wp.tile([64, N], F32, tag="s4")
                # rows 0-9: pa for cos; rows 32-41: pa again for sin.
                # Ranges extend over the zero pad rows so no SBUF garbage
                # (NaN) survives into qk (0 * NaN = NaN).
                nc.scalar.activation(s2[0:64, :], ps[0:64, :], AF.Sin, scale=0.5)
                nc.scalar.activation(s4[32:64, :], ps[32:64, :], AF.Sin,
                                     scale=0.25)
                sq2 = wp.tile([64, N], F32, tag="sq2")
                nc.vector.tensor_tensor(sq2[0:32, :], s2[0:32, :], s2[0:32, :],
                                        op=OP.mult)
                # cos rows -> qk[0:10] (pad rows get 1.0, masked out later)
                nc.vector.tensor_scalar(qk[0:32, :], sq2[0:32, :], -2.0, 1.0,
                                        op0=OP.mult, op1=OP.add)
                # sin rows: c4 = 1-2*s4^2 ; sin = 2*s2*c4 -> qk[32:42]
                sq4 = wp.tile([64, N], F32, tag="sq4")
                nc.vector.tensor_tensor(sq4[32:64, :], s4[32:64, :],
                                        s4[32:64, :], op=OP.mult)
                c4 = wp.tile([64, N], F32, tag="c4")
                nc.vector.tensor_scalar(c4[32:64, :], sq4[32:64, :], -2.0, 1.0,
                                        op0=OP.mult, op1=OP.add)
                nc.vector.scalar_tensor_tensor(qk[32:64, :], s2[32:64, :], 2.0,
                                               c4[32:64, :], op0=OP.mult,
                                               op1=OP.mult)
                # g rows
                nc.vector.tensor_copy(qk[64:QKR, :], ps[64:QKR, :])
                return qk

            QT, KT = [], []
            for b in range(B_LOC):
                QT.append(fan_qk(b, wq, "QT"))
                KT.append(fan_qk(b, wk, "KT"))

            # ---- phase V: natural orientation, fp8 vaug with ones cols ----
            # vaug_all[b]: [128, 4*48]; chunk c cols 48c..48c+47; head h col
            # 48c+6h+j = v ch 5h+j ; col 48c+6h+5 = 1.0
            vaug = []
            for b in range(B_LOC):
                va = pp.tile([128, 4 * 48], FP8, tag="vaug")
                nc.vector.memset(va[:], 1.0)
                for c in range(4):
                    ps = psS.tile([128, 30], F32, tag="fan")
                    nc.tensor.matmul(ps[:], xta[b][:, 128 * c:128 * (c + 1)],
                                     wv[:], start=True, stop=True)
                    s2v = wp.tile([128, 10], F32, tag="s2v")
                    s4v = wp.tile([128, 10], F32, tag="s4v")
                    nc.scalar.activation(s2v[:], ps[:, 0:10], AF.Sin, scale=0.5)
                    nc.scalar.activation(s4v[:], ps[:, 0:10], AF.Sin, scale=0.25)
                    sq2v = wp.tile([128, 10], F32, tag="sq2v")
                    nc.vector.tensor_tensor(sq2v[:], s2v[:], s2v[:], op=OP.mult)
                    # cos ch 0-9 -> heads 0,1 cols {48c+6h+j}
                    cosv = va[:, 48 * c:48 * c + 12].rearrange(
                        "p (h j) -> p h j", h=2)[:, :, 0:5]
                    nc.vector.tensor_scalar(
                        cosv, sq2v[:].rearrange("p (h j) -> p h j", h=2),
                        -2.0, 1.0, op0=OP.mult, op1=OP.add)
                    sq4v = wp.tile([128, 10], F32, tag="sq4v")
                    c4v = wp.tile([128, 10], F32, tag="c4v")
                    nc.vector.tensor_tensor(sq4v[:], s4v[:], s4v[:], op=OP.mult)
                    nc.vector.tensor_scalar(c4v[:], sq4v[:], -2.0, 1.0,
                                            op0=OP.mult, op1=OP.add)
                    sinv = wp.tile([128, 10], F32, tag="sinv")
                    nc.vector.scalar_tensor_tensor(sinv[:], s2v[:], 2.0, c4v[:],
                                                   op0=OP.mult, op1=OP.mult)
                    sin_dst = va[:, 48 * c + 12:48 * c + 24].rearrange(
                        "p (h j) -> p h j", h=2)[:, :, 0:5]
                    nc.vector.tensor_copy(
                        sin_dst, sinv[:].rearrange("p (h j) -> p h j", h=2))
                    g_dst = va[:, 48 * c + 24:48 * c + 48].rearrange(
                        "p (h j) -> p h j", h=4)[:, :, 0:5]
                    nc.vector.tensor_copy(
                        g_dst, ps[:, 10:30].rearrange("p (h j) -> p h j", h=4))
                vaug.append(va)

            # ---- phase W: gate scalars w1/w2 + diag tiles ----
            # Big warm block BEFORE the gate matmuls: the PE queue is
            # in-order, and the gates block on the fan DVE chains. These
            # independent matmuls execute during that wait, keeping the PE
            # continuously busy so HAM un-throttles early and stays at K=8.
            pe_warm(60)
            dgws = [[[None] * 4, [None] * 4] for _ in range(B_LOC)]
            for b in range(B_LOC):
                for wsel in range(2):
                    for blk in range(4):
                        sl = slice(128 * blk, 128 * (blk + 1))
                        zp = psS.tile([128, 1], F32, tag="fan")
                        nc.tensor.matmul(zp[:], QT[b][:, sl],
                                         dgw80[:, wsel:wsel + 1],
                                         start=True, stop=False)
                        nc.tensor.matmul(zp[:], ones_row[:],
                                         dgb[:, wsel:wsel + 1],
                                         start=False, stop=True)
                        th = wcp.tile([128, 1], F32, tag="th")
                        nc.scalar.activation(th[:], zp[:], AF.Tanh, scale=0.5)
                        wc = wcp.tile([128, 1], F32, tag="wcol")
                        nc.vector.tensor_scalar(wc[:], th[:], 0.5, 0.5,
                                                op0=OP.mult, op1=OP.add)
                        dw = wcp.tile([128, 128], BF16, tag="dgw")
                        nc.vector.tensor_scalar(dw[:], i128[:], wc[:], None,
                                                op0=OP.mult)
                        dgws[b][wsel][blk] = dw

            # ---- main loop ----
            # Software-pipelined: out-matmuls trail the energy/bias stream by
            # LAG (h,j)-groups so the PE never stalls waiting for an exp.
            # Stage5 of batch b-1 and kTm masks of batch b+1 are emitted into
            # the middle of batch b's stream to keep all queues busy.
            LAG = 2
            kTm = [None] * B_LOC

            def emit_masks(b):
                kTm[b] = []
                for h in range(H):
                    km = kmp.tile([QKR, N], BF16, tag=f"kTm{h}")
                    nc.vector.tensor_scalar(km[:], KT[b][:],
                                            masks[:, h:h + 1], None,
                                            op0=OP.mult)
                    kTm[b].append(km)

            def emit_stage5(b, out_lo, out_hi):
                sb_lo = wp.tile([128, N], BF16, tag="sb_lo")
                sb_hi = wp.tile([128, N], BF16, tag="sb_hi")
                nc.vector.tensor_copy(sb_lo[:], out_lo[:])
                nc.vector.tensor_copy(sb_hi[:], out_hi[:])
                sums8 = psS.tile([128, N], F32, tag="fan")
                nc.tensor.matmul(sums8[0:8, :], sel_lo[:], sb_lo[:],
                                 start=True, stop=False)
                nc.tensor.matmul(sums8[0:8, :], sel_hi[:], sb_hi[:],
                                 start=False, stop=True)
                recipf = wp.tile([8, N], F32, tag="recipf")
                nc.vector.reciprocal_approx_fast(recipf[:], sums8[0:8, :])
                recip8 = wp.tile([8, N], BF16, tag="recip8")
                nc.vector.tensor_copy(recip8[:], recipf[:])
                rm_lo = psS.tile([128, N], F32, tag="fan")
                nc.tensor.matmul(rm_lo[:], e5_lo[:], recip8[:],
                                 start=True, stop=True)
                sbn_lo = wp.tile([128, N], BF16, tag="sbn_lo")
                nc.vector.tensor_tensor(sbn_lo[:], sb_lo[:], rm_lo[:],
                                        op=OP.mult)
                rm_hi = psS.tile([128, N], F32, tag="fan")
                nc.tensor.matmul(rm_hi[:], e5_hi[:], recip8[:],
                                 start=True, stop=True)
                sbn_hi = wp.tile([128, N], BF16, tag="sbn_hi")
                nc.vector.tensor_tensor(sbn_hi[:], sb_hi[:], rm_hi[:],
                                        op=OP.mult)
                prj = psS.tile([128, N], F32, tag="fan")
                nc.tensor.matmul(prj[0:E, :], p_lo[:], sbn_lo[:],
                                 start=True, stop=False)
                nc.tensor.matmul(prj[0:E, :], p_hi[:], sbn_hi[:],
                                 start=False, stop=True)
                out_sb = wp.tile([E, N], F32, tag="out_sb")
                nc.scalar.activation(out_sb[:], prj[0:E, :], AF.Identity,
                                     bias=projb[:])
                nc.sync.dma_start(out_d[b][:], out_sb[:])

            def pop_out(entry):
                ops_, ob_, b_, h_, pair_, att2_ = entry
                for half in range(2):
                    j_ = 2 * pair_ + half
                    nc.tensor.matmul(
                        ops_[ob_:ob_ + 6, :],
                        vaug[b_][:, 48 * j_ + 6 * h_:48 * j_ + 6 * h_ + 6],
                        att2_[:, N * half:N * (half + 1)],
                        start=(j_ == 0), stop=(j_ == 3),
                        tile_position=(0, ob_), skip_group_check=True)

            emit_masks(0)
            if B_LOC > 1:
                emit_masks(1)
            outq = []
            prev_out = None
            for b in range(B_LOC):
                out_lo = psO.tile([128, N], F32, tag="out_lo")
                out_hi = psO.tile([128, N], F32, tag="out_hi")
                for h in range(H):
                    wsel = 0 if h < 4 else 1
                    out_ps = out_lo if h < 4 else out_hi
                    obase = 32 * (h % 4)
                    for pair in range(2):
                        if b == 0 and h < 4:
                            # keep the PE continuously busy while the Act
                            # queue drains the fan sins ahead of the first
                            # exps (HAM un-throttle needs ~3.4us busy)
                            pe_warm(4)
                        eT2 = psE.tile([128, 2 * N], F32, tag="eT")
                        for half in range(2):
                            j = 2 * pair + half
                            sl = slice(N * half, N * (half + 1))
                            nc.tensor.matmul(
                                eT2[:, sl],
                                kTm[b][h][:, 128 * j:128 * (j + 1)],
                                QT[b][:], start=True, stop=False,
                                skip_group_check=True)
                            for i in range(4):
                                nc.tensor.matmul(
                                    eT2[:, N * half + 128 * i:
                                        N * half + 128 * (i + 1)],
                                    dgt[b][h][:, 512 * i + 128 * j:
                                              512 * i + 128 * j + 128],
                                    dgws[b][wsel][i][:],
                                    start=False, stop=(i == 3),
                                    skip_group_check=True)
                        att2 = attp.tile([128, 2 * N], FP8, tag="att2")
                        nc.scalar.activation(att2[:], eT2[:], AF.Exp,
                                             scale=SCALE)
                        outq.append((out_ps, obase, b, h, pair, att2))
                        while len(outq) > LAG:
                            pop_out(outq.pop(0))
                    if h == 1:
                        if b + 2 < B_LOC:
                            emit_masks(b + 2)
                    if h == 2 and prev_out is not None:
                        emit_stage5(b - 1, prev_out[0], prev_out[1])
                        prev_out = None
                prev_out = (out_lo, out_hi)
            while outq:
                pop_out(outq.pop(0))
            emit_stage5(B_LOC - 1, prev_out[0], prev_out[1])

    lp.__exit__(None, None, None)
    nc.compile()
    _PROG_CACHE["nc"] = nc
    return nc


def _host_arrays(inputs):
    import ml_dtypes
    bf16 = ml_dtypes.bfloat16
    f32 = np.float32
    x = np.ascontiguousarray(inputs["x"], dtype=f32)
    ones = np.ones((B, 1, N), f32)
    xta = np.ascontiguousarray(
        np.concatenate([x.transpose(0, 2, 1), ones], axis=1)).astype(bf16)

    def aug(wp_, bp_):
        return np.concatenate([wp_, bp_[None, :]], 0).astype(f32)

    consts = {}
    qp = aug(inputs["q_Wp"], inputs["q_bp"])
    kp = aug(inputs["k_Wp"], inputs["k_bp"])
    qg = aug(inputs["q_Wg"], inputs["q_bg"])
    kg = aug(inputs["k_Wg"], inputs["k_bg"])
    # cols per proj: pa(cos) 0-9 | pad 10-31 | pa(sin) 32-41 | pad 42-63 |
    # g 64-83  (pads keep 32-aligned partition groups)
    z22 = np.zeros((41, 22), f32)
    consts["wq"] = np.ascontiguousarray(
        np.concatenate([qp, z22, qp, z22, qg], axis=1)).astype(bf16)
    consts["wk"] = np.ascontiguousarray(
        np.concatenate([kp, z22, kp, z22, kg], axis=1)).astype(bf16)
    vp = aug(inputs["v_Wp"], inputs["v_bp"])
    vg = aug(inputs["v_Wg"], inputs["v_bg"])
    consts["wv"] = np.ascontiguousarray(
        np.concatenate([vp, vg], axis=1)).astype(bf16)
    # gate vectors padded to the QK row layout
    dgw80 = np.zeros((QKR, 2), f32)
    dgw80[0:10, 0] = inputs["dg1_W"][0:10, 0]
    dgw80[32:42, 0] = inputs["dg1_W"][10:20, 0]
    dgw80[64:84, 1] = inputs["dg2_W"][:, 0]
    consts["dgw80"] = dgw80.astype(bf16)
    masks = np.zeros((QKR, H), f32)
    for h in range(H):
        r = R_CH[h]
        masks[r:r + 5, h] = 1.0
    consts["masks"] = masks
    consts["dgb"] = np.array([[inputs["dg1_b"][0], inputs["dg2_b"][0]]], bf16)
    sel_lo = np.zeros((128, 8), bf16)
    sel_hi = np.zeros((128, 8), bf16)
    e5_lo = np.zeros((8, 128), bf16)
    e5_hi = np.zeros((8, 128), bf16)
    p_lo = np.zeros((128, E), bf16)
    p_hi = np.zeros((128, E), bf16)
    for k in range(4):
        sel_lo[32 * k + 5, k] = 1.0
        sel_hi[32 * k + 5, 4 + k] = 1.0
        for j in range(5):
            e5_lo[k, 32 * k + j] = 1.0
            e5_hi[4 + k, 32 * k + j] = 1.0
            p_lo[32 * k + j, :] = inputs["proj_W"][5 * k + j, :]
            p_hi[32 * k + j, :] = inputs["proj_W"][20 + 5 * k + j, :]
    consts.update(sel_lo=sel_lo, sel_hi=sel_hi, e5_lo=e5_lo, e5_hi=e5_hi,
                  p_lo=p_lo, p_hi=p_hi)
    consts["projb"] = np.ascontiguousarray(
        inputs["proj_b"].astype(f32).reshape(E, 1))
    consts["i128"] = np.eye(128, dtype=f32).astype(bf16)
    return xta, consts


def kernel(**inputs):
    from concourse.bass_utils import run_bass_kernel_spmd
    import ml_dtypes

    nc = _build_program()
    xta, consts = _host_arrays(inputs)
    fp8 = ml_dtypes.float8_e4m3fn
    dg1 = np.ascontiguousarray(inputs["dynamic_graph1"]).astype(fp8)
    dg2 = np.ascontiguousarray(inputs["dynamic_graph2"]).astype(fp8)
    in_maps = []
    for c in range(NCORES):
        sl = slice(c * B_LOC, (c + 1) * B_LOC)
        m = {"xta": xta[sl], "dg1": dg1[sl], "dg2": dg2[sl]}
        m.update(consts)
        in_maps.append(m)
    res = run_bass_kernel_spmd(nc, in_maps, list(range(NCORES)))
    outT = np.concatenate([res.results[c]["outT"] for c in range(NCORES)], 0)
    return np.ascontiguousarray(outT.transpose(0, 2, 1)).astype(np.float32)


# revision 30
# speedup vs baseline: 1.0991x; 1.0991x over previous
"""Trainium2 Bass kernel for nn_FAA_51367808860389 (FAN-attention w/ dynamic-graph bias).

Strategy: data-parallel over batch B=32 across 8 cores (4 batches/core).
Everything computed in transposed orientation energyT[k,q]:
  - FAN q/k projections via ONE matmul per batch (stationary [41,80] with
    pa duplicated so cos- and sin-source rows land on distinct partitions),
    compact 80-row layout, per-head channel rows contiguous -> no masking.
  - energyT[k,q] per (head, k-block): K=5 matmul from row slices.
  - bias via diagonal-matmul: eT block += dg-block^T @ diag(w-block); dg
    streamed from HBM in fp8e4m3 (half DMA), one DMA per (batch, head).
  - exp on Act engine writes att directly in fp8; out-matmul uses fp8
    DoubleRow (pairs of k-blocks contracted per pass) with an appended
    ones column for the softmax denominators.
  - normalize + head-concat + out-projection fused via prepacked matrices.
Output produced transposed [40, 512] per batch; host transposes back.
"""
import numpy as np

B, N, E, H, D = 32, 512, 40, 8, 5
NCORES = 8
B_LOC = B // NCORES
SCALE = 1.0 / float(np.float32(E) ** 0.5)

# qT/kT row layout (identical for both): cos ch0-9 at rows 0-9 | pad 10-31 |
# sin ch10-19 at rows 32-41 | pad 42-63 | g ch20-39 at rows 64-83.
# 32-aligned groups (Act/DVE/matmul partition bases must be multiples of 32).
QKR = 84
R_CH = {h: (5 * h if h < 2 else (22 + 5 * h if h < 4 else 44 + 5 * h))
        for h in range(H)}

_PROG_CACHE = {}


def _build_program():
    if "nc" in _PROG_CACHE:
        return _PROG_CACHE["nc"]
    import concourse.bass as bass
    import concourse.tile as tile
    from concourse import bacc, mybir

    F32 = mybir.dt.float32
    BF16 = mybir.dt.bfloat16
    FP8 = mybir.dt.float8e4
    AF = mybir.ActivationFunctionType
    OP = mybir.AluOpType
    DR = mybir.MatmulPerfMode.DoubleRow

    nc = bacc.Bacc(None)
    dp = nc.declare_dram_parameter
    xta_d = dp("xta", [B_LOC, 41, N], BF16, isOutput=False)
    dg1_d = dp("dg1", [B_LOC, 4, N, N], FP8, isOutput=False)
    dg2_d = dp("dg2", [B_LOC, 4, N, N], FP8, isOutput=False)
    wq_d = dp("wq", [41, QKR], BF16, isOutput=False)
    wk_d = dp("wk", [41, QKR], BF16, isOutput=False)
    wv_d = dp("wv", [41, 30], BF16, isOutput=False)     # v fan weights
    dgw80_d = dp("dgw80", [QKR, 2], BF16, isOutput=False)
    masks_d = dp("masks", [QKR, H], F32, isOutput=False)
    dgb_d = dp("dgb", [1, 2], BF16, isOutput=False)
    sel_lo_d = dp("sel_lo", [128, 8], BF16, isOutput=False)
    sel_hi_d = dp("sel_hi", [128, 8], BF16, isOutput=False)
    e5_lo_d = dp("e5_lo", [8, 128], BF16, isOutput=False)
    e5_hi_d = dp("e5_hi", [8, 128], BF16, isOutput=False)
    p_lo_d = dp("p_lo", [128, E], BF16, isOutput=False)
    p_hi_d = dp("p_hi", [128, E], BF16, isOutput=False)
    projb_d = dp("projb", [E, 1], F32, isOutput=False)
    i128_d = dp("i128", [128, 128], BF16, isOutput=False)
    out_d = dp("outT", [B_LOC, E, N], F32, isOutput=True)

    lp = nc.allow_low_precision(reason="bf16/fp8 datapath validated vs reference")
    lp.__enter__()
    with tile.TileContext(nc) as tc:
        with (
            tc.tile_pool(name="const", bufs=1) as cp,
            tc.tile_pool(name="work", bufs=2) as wp,
            tc.tile_pool(name="persist", bufs=B_LOC) as pp,
            tc.tile_pool(name="wcolp", bufs=8 * B_LOC) as wcp,
            tc.tile_pool(name="dgp", bufs=B_LOC) as dgpool,
            tc.tile_pool(name="kmp", bufs=2) as kmp,
            tc.tile_pool(name="attp", bufs=6) as attp,
            tc.tile_pool(name="psE", bufs=2, space=bass.MemorySpace.PSUM) as psE,
            tc.tile_pool(name="psO", bufs=1, space=bass.MemorySpace.PSUM) as psO,
            tc.tile_pool(name="psS", bufs=2, space=bass.MemorySpace.PSUM) as psS,
        ):
            # ---- constants to SBUF ----
            def cload(dram, shape, tag, dt=F32):
                t = cp.tile(shape, dt, tag=tag)
                nc.sync.dma_start(t[:], dram[:])
                return t

            wq = cload(wq_d, [41, QKR], "wq", BF16)
            wk = cload(wk_d, [41, QKR], "wk", BF16)
            wv = cload(wv_d, [41, 30], "wv", BF16)
            dgw80 = cload(dgw80_d, [QKR, 2], "dgw80", BF16)
            masks = cload(masks_d, [QKR, H], "masks")
            dgb = cload(dgb_d, [1, 2], "dgb", BF16)
            sel_lo = cload(sel_lo_d, [128, 8], "sel_lo", BF16)
            sel_hi = cload(sel_hi_d, [128, 8], "sel_hi", BF16)
            e5_lo = cload(e5_lo_d, [8, 128], "e5_lo", BF16)
            e5_hi = cload(e5_hi_d, [8, 128], "e5_hi", BF16)
            p_lo = cload(p_lo_d, [128, E], "p_lo", BF16)
            p_hi = cload(p_hi_d, [128, E], "p_hi", BF16)
            projb = cload(projb_d, [E, 1], "projb")
            i128 = cload(i128_d, [128, 128], "i128", BF16)
            ones_row = cp.tile([1, 128], BF16, tag="ones_row")
            nc.vector.memset(ones_row[:], 1.0)
            # PE warm-up: HAM throttles the PE to half clock until it sees
            # ~3us of continuous matmul activity. Burn a few back-to-back
            # dummy matmuls at the start (and between phases) so the whole
            # main loop runs at full clock.
            warm_src = cp.tile([128, N], BF16, tag="warm_src")
            nc.vector.memset(warm_src[:], 0.0)

            def pe_warm(k, pool=None):
                # HAM only un-throttles the PE on sustained full-array
                # matmul activity, so warms must be real M=128 matmuls.
                # Phase warms write the psO bank (dead until the first real
                # out-matmul, which has start=True); main-loop warms use the
                # psS fan bank (idle there). WAW chains keep them PE-only.
                p = pool or psO
                tag = "out_lo" if p is psO else "fan"
                for _ in range(k):
                    pw = p.tile([128, N], F32, tag=tag)
                    nc.tensor.matmul(pw[:], i128[:], warm_src[:],
                                     start=True, stop=True)

            pe_warm(8)

            # ---- xta loads + dg prefetch (all upfront) ----
            xta = []
            for b in range(B_LOC):
                xt = pp.tile([41, N], BF16, tag="xta")
                nc.sync.dma_start(xt[:], xta_d[b][:])
                xta.append(xt)
            # dg tiles: [128, 2048] fp8, free = r*512 + n (r = q-row-block)
            dgt = [[None] * H for _ in range(B_LOC)]
            for b in range(B_LOC):
                for h in range(H):
                    src = dg1_d if h < 4 else dg2_d
                    t = dgpool.tile([128, 4 * N], FP8, tag=f"dg{h}")
                    nc.sync.dma_start(
                        t[:].rearrange("p (r n) -> p r n", r=4),
                        src[b, h % 4].rearrange("(r p) n -> p r n", p=128))
                    dgt[b][h] = t

            # ---- phase F: FAN q/k for all local batches ----
            def fan_qk(b, w, tag):
                ps = psS.tile([QKR, N], F32, tag="fan")
                nc.tensor.matmul(ps[:], w[:], xta[b][:], start=True, stop=True)
                qk = pp.tile([QKR, N], BF16, tag=tag)
                s2 = wp.tile([64, N], F32, tag="s2")
                s4 = wp.tile([64, N], F32, tag="s4")
                # rows 0-9: pa for cos; rows 32-41: pa again for sin.
                # Ranges extend over the zero pad rows so no SBUF garbage
                # (NaN) survives into qk (0 * NaN = NaN).
                nc.scalar.activation(s2[0:64, :], ps[0:64, :], AF.Sin, scale=0.5)
                nc.scalar.activation(s4[32:64, :], ps[32:64, :], AF.Sin,
                                     scale=0.25)
                sq2 = wp.tile([64, N], F32, tag="sq2")
                nc.vector.tensor_tensor(sq2[0:32, :], s2[0:32, :], s2[0:32, :],
                                        op=OP.mult)
                # cos rows -> qk[0:10] (pad rows get 1.0, masked out later)
                nc.vector.tensor_scalar(qk[0:32, :], sq2[0:32, :], -2.0, 1.0,
                                        op0=OP.mult, op1=OP.add)
                # sin rows: c4 = 1-2*s4^2 ; sin = 2*s2*c4 -> qk[32:42]
                sq4 = wp.tile([64, N], F32, tag="sq4")
                nc.vector.tensor_tensor(sq4[32:64, :], s4[32:64, :],
                                        s4[32:64, :], op=OP.mult)
                c4 = wp.tile([64, N], F32, tag="c4")
                nc.vector.tensor_scalar(c4[32:64, :], sq4[32:64, :], -2.0, 1.0,
                                        op0=OP.mult, op1=OP.add)
                nc.vector.scalar_tensor_tensor(qk[32:64, :], s2[32:64, :], 2.0,
                                               c4[32:64, :], op0=OP.mult,
                                               op1=OP.mult)
                # g rows
                nc.vector.tensor_copy(qk[64:QKR, :], ps[64:QKR, :])
                return qk

            QT, KT = [], []
            for b in range(B_LOC):
                QT.append(fan_qk(b, wq, "QT"))
                KT.append(fan_qk(b, wk, "KT"))

            # ---- phase V: natural orientation, fp8 vaug with ones cols ----
            # vaug_all[b]: [128, 4*48]; chunk c cols 48c..48c+47; head h col
            # 48c+6h+j = v ch 5h+j ; col 48c+6h+5 = 1.0
            vaug = []
            for b in range(B_LOC):
                va = pp.tile([128, 4 * 48], FP8, tag="vaug")
                nc.vector.memset(va[:], 1.0)
                for c in range(4):
                    ps = psS.tile([128, 30], F32, tag="fan")
                    nc.tensor.matmul(ps[:], xta[b][:, 128 * c:128 * (c + 1)],
                                     wv[:], start=True, stop=True)
                    s2v = wp.tile([128, 10], F32, tag="s2v")
                    s4v = wp.tile([128, 10], F32, tag="s4v")
                    nc.scalar.activation(s2v[:], ps[:, 0:10], AF.Sin, scale=0.5)
                    nc.scalar.activation(s4v[:], ps[:, 0:10], AF.Sin, scale=0.25)
                    sq2v = wp.tile([128, 10], F32, tag="sq2v")
                    nc.vector.tensor_tensor(sq2v[:], s2v[:], s2v[:], op=OP.mult)
                    # cos ch 0-9 -> heads 0,1 cols {48c+6h+j}
                    cosv = va[:, 48 * c:48 * c + 12].rearrange(
                        "p (h j) -> p h j", h=2)[:, :, 0:5]
                    nc.vector.tensor_scalar(
                        cosv, sq2v[:].rearrange("p (h j) -> p h j", h=2),
                        -2.0, 1.0, op0=OP.mult, op1=OP.add)
                    sq4v = wp.tile([128, 10], F32, tag="sq4v")
                    c4v = wp.tile([128, 10], F32, tag="c4v")
                    nc.vector.tensor_tensor(sq4v[:], s4v[:], s4v[:], op=OP.mult)
                    nc.vector.tensor_scalar(c4v[:], sq4v[:], -2.0, 1.0,
                                            op0=OP.mult, op1=OP.add)
                    sinv = wp.tile([128, 10], F32, tag="sinv")
                    nc.vector.scalar_tensor_tensor(sinv[:], s2v[:], 2.0, c4v[:],
                                                   op0=OP.mult, op1=OP.mult)
                    sin_dst = va[:, 48 * c + 12:48 * c + 24].rearrange(
                        "p (h j) -> p h j", h=2)[:, :, 0:5]
                    nc.vector.tensor_copy(
                        sin_dst, sinv[:].rearrange("p (h j) -> p h j", h=2))
                    g_dst = va[:, 48 * c + 24:48 * c + 48].rearrange(
                        "p (h j) -> p h j", h=4)[:, :, 0:5]
                    nc.vector.tensor_copy(
                        g_dst, ps[:, 10:30].rearrange("p (h j) -> p h j", h=4))
                vaug.append(va)

            # ---- phase W: gate scalars w1/w2 + diag tiles ----
            # Big warm block BEFORE the gate matmuls: the PE queue is
            # in-order, and the gates block on the fan DVE chains. These
            # independent matmuls execute during that wait, keeping the PE
            # continuously busy so HAM un-throttles early and stays at K=8.
            pe_warm(60)
            dgws = [[[None] * 4, [None] * 4] for _ in range(B_LOC)]
            for b in range(B_LOC):
                for wsel in range(2):
                    for blk in range(4):
                        sl = slice(128 * blk, 128 * (blk + 1))
                        zp = psS.tile([128, 1], F32, tag="fan")
                        nc.tensor.matmul(zp[:], QT[b][:, sl],
                                         dgw80[:, wsel:wsel + 1],
                                         start=True, stop=False)
                        nc.tensor.matmul(zp[:], ones_row[:],
                                         dgb[:, wsel:wsel + 1],
                                         start=False, stop=True)
                        th = wcp.tile([128, 1], F32, tag="th")
                        nc.scalar.activation(th[:], zp[:], AF.Tanh, scale=0.5)
                        wc = wcp.tile([128, 1], F32, tag="wcol")
                        nc.vector.tensor_scalar(wc[:], th[:], 0.5, 0.5,
                                                op0=OP.mult, op1=OP.add)
                        dw = wcp.tile([128, 128], BF16, tag="dgw")
                        nc.vector.tensor_scalar(dw[:], i128[:], wc[:], None,
                                                op0=OP.mult)
                        dgws[b][wsel][blk] = dw

            # ---- main loop ----
            # Software-pipelined: out-matmuls trail the energy/bias stream by
            # LAG (h,j)-groups so the PE never stalls waiting for an exp.
            # Stage5 of batch b-1 and kTm masks of batch b+1 are emitted into
            # the middle of batch b's stream to keep all queues busy.
            LAG = 2
            kTm = [None] * B_LOC

            def emit_masks(b):
                kTm[b] = []
                for h in range(H):
                    km = kmp.tile([QKR, N], BF16, tag=f"kTm{h}")
                    nc.vector.tensor_scalar(km[:], KT[b][:],
                                            masks[:, h:h + 1], None,
                                            op0=OP.mult)
                    kTm[b].append(km)

            def emit_stage5(b, out_lo, out_hi):
                sb_lo = wp.tile([128, N], BF16, tag="sb_lo")
                sb_hi = wp.tile([128, N], BF16, tag="sb_hi")
                nc.vector.tensor_copy(sb_lo[:], out_lo[:])
                nc.vector.tensor_copy(sb_hi[:], out_hi[:])
                sums8 = psS.tile([128, N], F32, tag="fan")
                nc.tensor.matmul(sums8[0:8, :], sel_lo[:], sb_lo[:],
                                 start=True, stop=False)
                nc.tensor.matmul(sums8[0:8, :], sel_hi[:], sb_hi[:],
                                 start=False, stop=True)
                recipf = wp.tile([8, N], F32, tag="recipf")
                nc.vector.reciprocal_approx_fast(recipf[:], sums8[0:8, :])
                recip8 = wp.tile([8, N], BF16, tag="recip8")
                nc.vector.tensor_copy(recip8[:], recipf[:])
                rm_lo = psS.tile([128, N], F32, tag="fan")
                nc.tensor.matmul(rm_lo[:], e5_lo[:], recip8[:],
                                 start=True, stop=True)
                sbn_lo = wp.tile([128, N], BF16, tag="sbn_lo")
                nc.vector.tensor_tensor(sbn_lo[:], sb_lo[:], rm_lo[:],
                                        op=OP.mult)
                rm_hi = psS.tile([128, N], F32, tag="fan")
                nc.tensor.matmul(rm_hi[:], e5_hi[:], recip8[:],
                                 start=True, stop=True)
                sbn_hi = wp.tile([128, N], BF16, tag="sbn_hi")
                nc.vector.tensor_tensor(sbn_hi[:], sb_hi[:], rm_hi[:],
                                        op=OP.mult)
                prj = psS.tile([128, N], F32, tag="fan")
                nc.tensor.matmul(prj[0:E, :], p_lo[:], sbn_lo[:],
                                 start=True, stop=False)
                nc.tensor.matmul(prj[0:E, :], p_hi[:], sbn_hi[:],
                                 start=False, stop=True)
                out_sb = wp.tile([E, N], F32, tag="out_sb")
                nc.scalar.activation(out_sb[:], prj[0:E, :], AF.Identity,
                                     bias=projb[:])
                nc.sync.dma_start(out_d[b][:], out_sb[:])

            def pop_out(entry):
                ops_, ob_, b_, h_, pair_, att2_ = entry
                for half in range(2):
                    j_ = 2 * pair_ + half
                    nc.tensor.matmul(
                        ops_[ob_:ob_ + 6, :],
                        vaug[b_][:, 48 * j_ + 6 * h_:48 * j_ + 6 * h_ + 6],
                        att2_[:, N * half:N * (half + 1)],
                        start=(j_ == 0), stop=(j_ == 3),
                        tile_position=(0, ob_), skip_group_check=True)

            emit_masks(0)
            if B_LOC > 1:
                emit_masks(1)
            outq = []
            prev_out = None
            for b in range(B_LOC):
                out_lo = psO.tile([128, N], F32, tag="out_lo")
                out_hi = psO.tile([128, N], F32, tag="out_hi")
                for h in range(H):
                    wsel = 0 if h < 4 else 1
                    out_ps = out_lo if h < 4 else out_hi
                    obase = 32 * (h % 4)
                    for pair in range(2):
                        if b == 0 and h < 4:
                            # keep the PE continuously busy while the Act
                            # queue drains the fan sins ahead of the first
                            # exps (HAM un-throttle needs ~3.4us busy)
                            pe_warm(4, psS)
                        eT2 = psE.tile([128, 2 * N], F32, tag="eT")
                        for half in range(2):
                            j = 2 * pair + half
                            sl = slice(N * half, N * (half + 1))
                            nc.tensor.matmul(
                                eT2[:, sl],
                                kTm[b][h][:, 128 * j:128 * (j + 1)],
                                QT[b][:], start=True, stop=False,
                                skip_group_check=True)
                            for i in range(4):
                                nc.tensor.matmul(
                                    eT2[:, N * half + 128 * i:
                                        N * half + 128 * (i + 1)],
                                    dgt[b][h][:, 512 * i + 128 * j:
                                              512 * i + 128 * j + 128],
                                    dgws[b][wsel][i][:],
                                    start=False, stop=(i == 3),
                                    skip_group_check=True)
                        att2 = attp.tile([128, 2 * N], FP8, tag="att2")
                        nc.scalar.activation(att2[:], eT2[:], AF.Exp,
                                             scale=SCALE)
                        outq.append((out_ps, obase, b, h, pair, att2))
                        while len(outq) > LAG:
                            pop_out(outq.pop(0))
                    if h == 1:
                        if b + 2 < B_LOC:
                            emit_masks(b + 2)
                    if h == 2 and prev_out is not None:
                        emit_stage5(b - 1, prev_out[0], prev_out[1])
                        prev_out = None
                prev_out = (out_lo, out_hi)
            while outq:
                pop_out(outq.pop(0))
            emit_stage5(B_LOC - 1, prev_out[0], prev_out[1])

    lp.__exit__(None, None, None)
    nc.compile()
    _PROG_CACHE["nc"] = nc
    return nc


def _host_arrays(inputs):
    import ml_dtypes
    bf16 = ml_dtypes.bfloat16
    f32 = np.float32
    x = np.ascontiguousarray(inputs["x"], dtype=f32)
    ones = np.ones((B, 1, N), f32)
    xta = np.ascontiguousarray(
        np.concatenate([x.transpose(0, 2, 1), ones], axis=1)).astype(bf16)

    def aug(wp_, bp_):
        return np.concatenate([wp_, bp_[None, :]], 0).astype(f32)

    consts = {}
    qp = aug(inputs["q_Wp"], inputs["q_bp"])
    kp = aug(inputs["k_Wp"], inputs["k_bp"])
    qg = aug(inputs["q_Wg"], inputs["q_bg"])
    kg = aug(inputs["k_Wg"], inputs["k_bg"])
    # cols per proj: pa(cos) 0-9 | pad 10-31 | pa(sin) 32-41 | pad 42-63 |
    # g 64-83  (pads keep 32-aligned partition groups)
    z22 = np.zeros((41, 22), f32)
    consts["wq"] = np.ascontiguousarray(
        np.concatenate([qp, z22, qp, z22, qg], axis=1)).astype(bf16)
    consts["wk"] = np.ascontiguousarray(
        np.concatenate([kp, z22, kp, z22, kg], axis=1)).astype(bf16)
    vp = aug(inputs["v_Wp"], inputs["v_bp"])
    vg = aug(inputs["v_Wg"], inputs["v_bg"])
    consts["wv"] = np.ascontiguousarray(
        np.concatenate([vp, vg], axis=1)).astype(bf16)
    # gate vectors padded to the QK row layout
    dgw80 = np.zeros((QKR, 2), f32)
    dgw80[0:10, 0] = inputs["dg1_W"][0:10, 0]
    dgw80[32:42, 0] = inputs["dg1_W"][10:20, 0]
    dgw80[64:84, 1] = inputs["dg2_W"][:, 0]
    consts["dgw80"] = dgw80.astype(bf16)
    masks = np.zeros((QKR, H), f32)
    for h in range(H):
        r = R_CH[h]
        masks[r:r + 5, h] = 1.0
    consts["masks"] = masks
    consts["dgb"] = np.array([[inputs["dg1_b"][0], inputs["dg2_b"][0]]], bf16)
    sel_lo = np.zeros((128, 8), bf16)
    sel_hi = np.zeros((128, 8), bf16)
    e5_lo = np.zeros((8, 128), bf16)
    e5_hi = np.zeros((8, 128), bf16)
    p_lo = np.zeros((128, E), bf16)
    p_hi = np.zeros((128, E), bf16)
    for k in range(4):
        sel_lo[32 * k + 5, k] = 1.0
        sel_hi[32 * k + 5, 4 + k] = 1.0
        for j in range(5):
            e5_lo[k, 32 * k + j] = 1.0
            e5_hi[4 + k, 32 * k + j] = 1.0
            p_lo[32 * k + j, :] = inputs["proj_W"][5 * k + j, :]
            p_hi[32 * k + j, :] = inputs["proj_W"][20 + 5 * k + j, :]
    consts.update(sel_lo=sel_lo, sel_hi=sel_hi, e5_lo=e5_lo, e5_hi=e5_hi,
                  p_lo=p_lo, p_hi=p_hi)
    consts["projb"] = np.ascontiguousarray(
        inputs["proj_b"].astype(f32).reshape(E, 1))
    consts["i128"] = np.eye(128, dtype=f32).astype(bf16)
    return xta, consts


def kernel(**inputs):
    from concourse.bass_utils import run_bass_kernel_spmd
    import ml_dtypes

    nc = _build_program()
    xta, consts = _host_arrays(inputs)
    fp8 = ml_dtypes.float8_e4m3fn
    dg1 = np.ascontiguousarray(inputs["dynamic_graph1"]).astype(fp8)
    dg2 = np.ascontiguousarray(inputs["dynamic_graph2"]).astype(fp8)
    in_maps = []
    for c in range(NCORES):
        sl = slice(c * B_LOC, (c + 1) * B_LOC)
        m = {"xta": xta[sl], "dg1": dg1[sl], "dg2": dg2[sl]}
        m.update(consts)
        in_maps.append(m)
    res = run_bass_kernel_spmd(nc, in_maps, list(range(NCORES)))
    outT = np.concatenate([res.results[c]["outT"] for c in range(NCORES)], 0)
    return np.ascontiguousarray(outT.transpose(0, 2, 1)).astype(np.float32)


# revision 31
# speedup vs baseline: 1.1478x; 1.0444x over previous
"""Trainium2 Bass kernel for nn_FAA_51367808860389 (FAN-attention w/ dynamic-graph bias).

Strategy: data-parallel over batch B=32 across 8 cores (4 batches/core).
Everything computed in transposed orientation energyT[k,q]:
  - FAN q/k projections via ONE matmul per batch (stationary [41,80] with
    pa duplicated so cos- and sin-source rows land on distinct partitions),
    compact 80-row layout, per-head channel rows contiguous -> no masking.
  - energyT[k,q] per (head, k-block): K=5 matmul from row slices.
  - bias via diagonal-matmul: eT block += dg-block^T @ diag(w-block); dg
    streamed from HBM in fp8e4m3 (half DMA), one DMA per (batch, head).
  - exp on Act engine writes att directly in fp8; out-matmul uses fp8
    DoubleRow (pairs of k-blocks contracted per pass) with an appended
    ones column for the softmax denominators.
  - normalize + head-concat + out-projection fused via prepacked matrices.
Output produced transposed [40, 512] per batch; host transposes back.
"""
import numpy as np

B, N, E, H, D = 32, 512, 40, 8, 5
NCORES = 8
B_LOC = B // NCORES
SCALE = 1.0 / float(np.float32(E) ** 0.5)

# qT/kT row layout (identical for both): cos ch0-9 at rows 0-9 | pad 10-31 |
# sin ch10-19 at rows 32-41 | pad 42-63 | g ch20-39 at rows 64-83.
# 32-aligned groups (Act/DVE/matmul partition bases must be multiples of 32).
QKR = 84
R_CH = {h: (5 * h if h < 2 else (22 + 5 * h if h < 4 else 44 + 5 * h))
        for h in range(H)}

_PROG_CACHE = {}


def _build_program():
    if "nc" in _PROG_CACHE:
        return _PROG_CACHE["nc"]
    import concourse.bass as bass
    import concourse.tile as tile
    from concourse import bacc, mybir

    F32 = mybir.dt.float32
    BF16 = mybir.dt.bfloat16
    FP8 = mybir.dt.float8e4
    AF = mybir.ActivationFunctionType
    OP = mybir.AluOpType
    DR = mybir.MatmulPerfMode.DoubleRow

    nc = bacc.Bacc(None)
    dp = nc.declare_dram_parameter
    xta_d = dp("xta", [B_LOC, 41, N], BF16, isOutput=False)
    dg1_d = dp("dg1", [B_LOC, 4, N, N], FP8, isOutput=False)
    dg2_d = dp("dg2", [B_LOC, 4, N, N], FP8, isOutput=False)
    wq_d = dp("wq", [41, QKR], BF16, isOutput=False)
    wk_d = dp("wk", [41, QKR], BF16, isOutput=False)
    wv_d = dp("wv", [41, 30], BF16, isOutput=False)     # v fan weights
    dgw80_d = dp("dgw80", [QKR, 2], BF16, isOutput=False)
    masks_d = dp("masks", [QKR, H], F32, isOutput=False)
    dgb_d = dp("dgb", [1, 2], BF16, isOutput=False)
    sel_lo_d = dp("sel_lo", [128, 8], BF16, isOutput=False)
    sel_hi_d = dp("sel_hi", [128, 8], BF16, isOutput=False)
    e5_lo_d = dp("e5_lo", [8, 128], BF16, isOutput=False)
    e5_hi_d = dp("e5_hi", [8, 128], BF16, isOutput=False)
    p_lo_d = dp("p_lo", [128, E], BF16, isOutput=False)
    p_hi_d = dp("p_hi", [128, E], BF16, isOutput=False)
    projb_d = dp("projb", [E, 1], F32, isOutput=False)
    i128_d = dp("i128", [128, 128], BF16, isOutput=False)
    out_d = dp("outT", [B_LOC, E, N], F32, isOutput=True)

    lp = nc.allow_low_precision(reason="bf16/fp8 datapath validated vs reference")
    lp.__enter__()
    with tile.TileContext(nc) as tc:
        with (
            tc.tile_pool(name="const", bufs=1) as cp,
            tc.tile_pool(name="work", bufs=2) as wp,
            tc.tile_pool(name="persist", bufs=B_LOC) as pp,
            tc.tile_pool(name="wcolp", bufs=8 * B_LOC) as wcp,
            tc.tile_pool(name="dgp", bufs=B_LOC) as dgpool,
            tc.tile_pool(name="kmp", bufs=B_LOC) as kmp,
            tc.tile_pool(name="attp", bufs=6) as attp,
            tc.tile_pool(name="psE", bufs=2, space=bass.MemorySpace.PSUM) as psE,
            tc.tile_pool(name="psO", bufs=1, space=bass.MemorySpace.PSUM) as psO,
            tc.tile_pool(name="psS", bufs=2, space=bass.MemorySpace.PSUM) as psS,
        ):
            # ---- constants to SBUF ----
            def cload(dram, shape, tag, dt=F32):
                t = cp.tile(shape, dt, tag=tag)
                nc.sync.dma_start(t[:], dram[:])
                return t

            wq = cload(wq_d, [41, QKR], "wq", BF16)
            wk = cload(wk_d, [41, QKR], "wk", BF16)
            wv = cload(wv_d, [41, 30], "wv", BF16)
            dgw80 = cload(dgw80_d, [QKR, 2], "dgw80", BF16)
            masks = cload(masks_d, [QKR, H], "masks")
            dgb = cload(dgb_d, [1, 2], "dgb", BF16)
            sel_lo = cload(sel_lo_d, [128, 8], "sel_lo", BF16)
            sel_hi = cload(sel_hi_d, [128, 8], "sel_hi", BF16)
            e5_lo = cload(e5_lo_d, [8, 128], "e5_lo", BF16)
            e5_hi = cload(e5_hi_d, [8, 128], "e5_hi", BF16)
            p_lo = cload(p_lo_d, [128, E], "p_lo", BF16)
            p_hi = cload(p_hi_d, [128, E], "p_hi", BF16)
            projb = cload(projb_d, [E, 1], "projb")
            i128 = cload(i128_d, [128, 128], "i128", BF16)
            ones_row = cp.tile([1, 128], BF16, tag="ones_row")
            nc.vector.memset(ones_row[:], 1.0)
            # PE warm-up: HAM throttles the PE to half clock until it sees
            # ~3us of continuous matmul activity. Burn a few back-to-back
            # dummy matmuls at the start (and between phases) so the whole
            # main loop runs at full clock.
            warm_src = cp.tile([128, N], BF16, tag="warm_src")
            nc.vector.memset(warm_src[:], 0.0)

            def pe_warm(k, pool=None):
                # HAM only un-throttles the PE on sustained full-array
                # matmul activity, so warms must be real M=128 matmuls.
                # Phase warms write the psO bank (dead until the first real
                # out-matmul, which has start=True); main-loop warms use the
                # psS fan bank (idle there). WAW chains keep them PE-only.
                p = pool or psO
                tag = "out_lo" if p is psO else "fan"
                for _ in range(k):
                    pw = p.tile([128, N], F32, tag=tag)
                    nc.tensor.matmul(pw[:], i128[:], warm_src[:],
                                     start=True, stop=True)

            pe_warm(8)

            # ---- xta loads + dg prefetch (all upfront) ----
            xta = []
            for b in range(B_LOC):
                xt = pp.tile([41, N], BF16, tag="xta")
                nc.sync.dma_start(xt[:], xta_d[b][:])
                xta.append(xt)
            # dg tiles: [128, 2048] fp8, free = r*512 + n (r = q-row-block)
            dgt = [[None] * H for _ in range(B_LOC)]
            for b in range(B_LOC):
                for h in range(H):
                    src = dg1_d if h < 4 else dg2_d
                    t = dgpool.tile([128, 4 * N], FP8, tag=f"dg{h}")
                    nc.sync.dma_start(
                        t[:].rearrange("p (r n) -> p r n", r=4),
                        src[b, h % 4].rearrange("(r p) n -> p r n", p=128))
                    dgt[b][h] = t

            # ---- phase F: FAN q/k for all local batches ----
            def fan_qk(b, w, tag):
                ps = psS.tile([QKR, N], F32, tag="fan")
                nc.tensor.matmul(ps[:], w[:], xta[b][:], start=True, stop=True)
                qk = pp.tile([QKR, N], BF16, tag=tag)
                s2 = wp.tile([64, N], F32, tag="s2")
                s4 = wp.tile([64, N], F32, tag="s4")
                # rows 0-9: pa for cos; rows 32-41: pa again for sin.
                # Ranges extend over the zero pad rows so no SBUF garbage
                # (NaN) survives into qk (0 * NaN = NaN).
                nc.scalar.activation(s2[0:64, :], ps[0:64, :], AF.Sin, scale=0.5)
                nc.scalar.activation(s4[32:64, :], ps[32:64, :], AF.Sin,
                                     scale=0.25)
                sq2 = wp.tile([64, N], F32, tag="sq2")
                nc.vector.tensor_tensor(sq2[0:32, :], s2[0:32, :], s2[0:32, :],
                                        op=OP.mult)
                # cos rows -> qk[0:10] (pad rows get 1.0, masked out later)
                nc.vector.tensor_scalar(qk[0:32, :], sq2[0:32, :], -2.0, 1.0,
                                        op0=OP.mult, op1=OP.add)
                # sin rows: c4 = 1-2*s4^2 ; sin = 2*s2*c4 -> qk[32:42]
                sq4 = wp.tile([64, N], F32, tag="sq4")
                nc.vector.tensor_tensor(sq4[32:64, :], s4[32:64, :],
                                        s4[32:64, :], op=OP.mult)
                c4 = wp.tile([64, N], F32, tag="c4")
                nc.vector.tensor_scalar(c4[32:64, :], sq4[32:64, :], -2.0, 1.0,
                                        op0=OP.mult, op1=OP.add)
                nc.vector.scalar_tensor_tensor(qk[32:64, :], s2[32:64, :], 2.0,
                                               c4[32:64, :], op0=OP.mult,
                                               op1=OP.mult)
                # g rows
                nc.vector.tensor_copy(qk[64:QKR, :], ps[64:QKR, :])
                return qk

            kTm = [None] * B_LOC

            def emit_masks(b):
                kTm[b] = []
                for h in range(H):
                    km = kmp.tile([QKR, N], BF16, tag=f"kTm{h}")
                    nc.vector.tensor_scalar(km[:], KT[b][:],
                                            masks[:, h:h + 1], None,
                                            op0=OP.mult)
                    kTm[b].append(km)

            # batch-major so batch 0's DVE work sits first in the queue;
            # masks(b) follow their own batch's chain
            QT, KT = [None] * B_LOC, [None] * B_LOC
            for b in range(B_LOC):
                QT[b] = fan_qk(b, wq, "QT")
                KT[b] = fan_qk(b, wk, "KT")
                emit_masks(b)

            # ---- phase V: natural orientation, fp8 vaug with ones cols ----
            # vaug_all[b]: [128, 4*48]; chunk c cols 48c..48c+47; head h col
            # 48c+6h+j = v ch 5h+j ; col 48c+6h+5 = 1.0
            vaug = []
            for b in range(B_LOC):
                va = pp.tile([128, 4 * 48], FP8, tag="vaug")
                nc.vector.memset(va[:], 1.0)
                for c in range(4):
                    ps = psS.tile([128, 30], F32, tag="fan")
                    nc.tensor.matmul(ps[:], xta[b][:, 128 * c:128 * (c + 1)],
                                     wv[:], start=True, stop=True)
                    s2v = wp.tile([128, 10], F32, tag="s2v")
                    s4v = wp.tile([128, 10], F32, tag="s4v")
                    nc.scalar.activation(s2v[:], ps[:, 0:10], AF.Sin, scale=0.5)
                    nc.scalar.activation(s4v[:], ps[:, 0:10], AF.Sin, scale=0.25)
                    sq2v = wp.tile([128, 10], F32, tag="sq2v")
                    nc.vector.tensor_tensor(sq2v[:], s2v[:], s2v[:], op=OP.mult)
                    # cos ch 0-9 -> heads 0,1 cols {48c+6h+j}
                    cosv = va[:, 48 * c:48 * c + 12].rearrange(
                        "p (h j) -> p h j", h=2)[:, :, 0:5]
                    nc.vector.tensor_scalar(
                        cosv, sq2v[:].rearrange("p (h j) -> p h j", h=2),
                        -2.0, 1.0, op0=OP.mult, op1=OP.add)
                    sq4v = wp.tile([128, 10], F32, tag="sq4v")
                    c4v = wp.tile([128, 10], F32, tag="c4v")
                    nc.vector.tensor_tensor(sq4v[:], s4v[:], s4v[:], op=OP.mult)
                    nc.vector.tensor_scalar(c4v[:], sq4v[:], -2.0, 1.0,
                                            op0=OP.mult, op1=OP.add)
                    sinv = wp.tile([128, 10], F32, tag="sinv")
                    nc.vector.scalar_tensor_tensor(sinv[:], s2v[:], 2.0, c4v[:],
                                                   op0=OP.mult, op1=OP.mult)
                    sin_dst = va[:, 48 * c + 12:48 * c + 24].rearrange(
                        "p (h j) -> p h j", h=2)[:, :, 0:5]
                    nc.vector.tensor_copy(
                        sin_dst, sinv[:].rearrange("p (h j) -> p h j", h=2))
                    g_dst = va[:, 48 * c + 24:48 * c + 48].rearrange(
                        "p (h j) -> p h j", h=4)[:, :, 0:5]
                    nc.vector.tensor_copy(
                        g_dst, ps[:, 10:30].rearrange("p (h j) -> p h j", h=4))
                vaug.append(va)

            # ---- gates (tanh lives in the exp act table, so gates for
            # b>=1 are deferred into the main loop: no table thrash) ----
            dgws = [[[None] * 4, [None] * 4] for _ in range(B_LOC)]

            def emit_gates(b):
                for wsel in range(2):
                    for blk in range(4):
                        sl = slice(128 * blk, 128 * (blk + 1))
                        zp = psS.tile([128, 1], F32, tag="fan")
                        nc.tensor.matmul(zp[:], QT[b][:, sl],
                                         dgw80[:, wsel:wsel + 1],
                                         start=True, stop=False)
                        nc.tensor.matmul(zp[:], ones_row[:],
                                         dgb[:, wsel:wsel + 1],
                                         start=False, stop=True)
                        th = wcp.tile([128, 1], F32, tag="th")
                        nc.scalar.activation(th[:], zp[:], AF.Tanh, scale=0.5)
                        wc = wcp.tile([128, 1], F32, tag="wcol")
                        nc.vector.tensor_scalar(wc[:], th[:], 0.5, 0.5,
                                                op0=OP.mult, op1=OP.add)
                        dw = wcp.tile([128, 128], BF16, tag="dgw")
                        nc.vector.tensor_scalar(dw[:], i128[:], wc[:], None,
                                                op0=OP.mult)
                        dgws[b][wsel][blk] = dw

            # PE filler while the Act queue drains the sins ahead of the
            # first exps; executes during the wait, keeps HAM at K=8
            pe_warm(45)
            emit_gates(0)

            # ---- main loop ----
            # Software-pipelined: out-matmuls trail the energy/bias stream
            # by LAG pair-groups so the PE never stalls on an exp. Gates of
            # b+1 and stage5 of b-1 are folded into batch b's stream.
            LAG = 2

            def emit_stage5(b, out_lo, out_hi):
                sb_lo = wp.tile([128, N], BF16, tag="sb_lo")
                sb_hi = wp.tile([128, N], BF16, tag="sb_hi")
                nc.vector.tensor_copy(sb_lo[:], out_lo[:])
                nc.vector.tensor_copy(sb_hi[:], out_hi[:])
                sums8 = psS.tile([128, N], F32, tag="fan")
                nc.tensor.matmul(sums8[0:8, :], sel_lo[:], sb_lo[:],
                                 start=True, stop=False)
                nc.tensor.matmul(sums8[0:8, :], sel_hi[:], sb_hi[:],
                                 start=False, stop=True)
                recipf = wp.tile([8, N], F32, tag="recipf")
                nc.vector.reciprocal_approx_fast(recipf[:], sums8[0:8, :])
                recip8 = wp.tile([8, N], BF16, tag="recip8")
                nc.vector.tensor_copy(recip8[:], recipf[:])
                rm_lo = psS.tile([128, N], F32, tag="fan")
                nc.tensor.matmul(rm_lo[:], e5_lo[:], recip8[:],
                                 start=True, stop=True)
                sbn_lo = wp.tile([128, N], BF16, tag="sbn_lo")
                nc.vector.tensor_tensor(sbn_lo[:], sb_lo[:], rm_lo[:],
                                        op=OP.mult)
                rm_hi = psS.tile([128, N], F32, tag="fan")
                nc.tensor.matmul(rm_hi[:], e5_hi[:], recip8[:],
                                 start=True, stop=True)
                sbn_hi = wp.tile([128, N], BF16, tag="sbn_hi")
                nc.vector.tensor_tensor(sbn_hi[:], sb_hi[:], rm_hi[:],
                                        op=OP.mult)
                prj = psS.tile([128, N], F32, tag="fan")
                nc.tensor.matmul(prj[0:E, :], p_lo[:], sbn_lo[:],
                                 start=True, stop=False)
                nc.tensor.matmul(prj[0:E, :], p_hi[:], sbn_hi[:],
                                 start=False, stop=True)
                out_sb = wp.tile([E, N], F32, tag="out_sb")
                nc.scalar.activation(out_sb[:], prj[0:E, :], AF.Identity,
                                     bias=projb[:])
                nc.sync.dma_start(out_d[b][:], out_sb[:])

            def pop_out(entry):
                ops_, ob_, b_, h_, pair_, att2_ = entry
                for half in range(2):
                    j_ = 2 * pair_ + half
                    nc.tensor.matmul(
                        ops_[ob_:ob_ + 6, :],
                        vaug[b_][:, 48 * j_ + 6 * h_:48 * j_ + 6 * h_ + 6],
                        att2_[:, N * half:N * (half + 1)],
                        start=(j_ == 0), stop=(j_ == 3),
                        tile_position=(0, ob_), skip_group_check=True)

            outq = []
            prev_out = None
            for b in range(B_LOC):
                out_lo = psO.tile([128, N], F32, tag="out_lo")
                out_hi = psO.tile([128, N], F32, tag="out_hi")
                for h in range(H):
                    wsel = 0 if h < 4 else 1
                    out_ps = out_lo if h < 4 else out_hi
                    obase = 32 * (h % 4)
                    for pair in range(2):
                        eT2 = psE.tile([128, 2 * N], F32, tag="eT")
                        for half in range(2):
                            j = 2 * pair + half
                            sl = slice(N * half, N * (half + 1))
                            nc.tensor.matmul(
                                eT2[:, sl],
                                kTm[b][h][:, 128 * j:128 * (j + 1)],
                                QT[b][:], start=True, stop=False,
                                skip_group_check=True)
                            for i in range(4):
                                nc.tensor.matmul(
                                    eT2[:, N * half + 128 * i:
                                        N * half + 128 * (i + 1)],
                                    dgt[b][h][:, 512 * i + 128 * j:
                                              512 * i + 128 * j + 128],
                                    dgws[b][wsel][i][:],
                                    start=False, stop=(i == 3),
                                    skip_group_check=True)
                        att2 = attp.tile([128, 2 * N], FP8, tag="att2")
                        nc.scalar.activation(att2[:], eT2[:], AF.Exp,
                                             scale=SCALE)
                        outq.append((out_ps, obase, b, h, pair, att2))
                        while len(outq) > LAG:
                            pop_out(outq.pop(0))
                    if h == 1 and b + 1 < B_LOC:
                        emit_gates(b + 1)
                    if h == 2 and prev_out is not None:
                        emit_stage5(b - 1, prev_out[0], prev_out[1])
                        prev_out = None
                prev_out = (out_lo, out_hi)
            while outq:
                pop_out(outq.pop(0))
            emit_stage5(B_LOC - 1, prev_out[0], prev_out[1])

    lp.__exit__(None, None, None)
    nc.compile()
    _PROG_CACHE["nc"] = nc
    return nc


def _host_arrays(inputs):
    import ml_dtypes
    bf16 = ml_dtypes.bfloat16
    f32 = np.float32
    x = np.ascontiguousarray(inputs["x"], dtype=f32)
    ones = np.ones((B, 1, N), f32)
    xta = np.ascontiguousarray(
        np.concatenate([x.transpose(0, 2, 1), ones], axis=1)).astype(bf16)

    def aug(wp_, bp_):
        return np.concatenate([wp_, bp_[None, :]], 0).astype(f32)

    consts = {}
    qp = aug(inputs["q_Wp"], inputs["q_bp"])
    kp = aug(inputs["k_Wp"], inputs["k_bp"])
    qg = aug(inputs["q_Wg"], inputs["q_bg"])
    kg = aug(inputs["k_Wg"], inputs["k_bg"])
    # cols per proj: pa(cos) 0-9 | pad 10-31 | pa(sin) 32-41 | pad 42-63 |
    # g 64-83  (pads keep 32-aligned partition groups)
    z22 = np.zeros((41, 22), f32)
    consts["wq"] = np.ascontiguousarray(
        np.concatenate([qp, z22, qp, z22, qg], axis=1)).astype(bf16)
    consts["wk"] = np.ascontiguousarray(
        np.concatenate([kp, z22, kp, z22, kg], axis=1)).astype(bf16)
    vp = aug(inputs["v_Wp"], inputs["v_bp"])
    vg = aug(inputs["v_Wg"], inputs["v_bg"])
    consts["wv"] = np.ascontiguousarray(
        np.concatenate([vp, vg], axis=1)).astype(bf16)
    # gate vectors padded to the QK row layout
    dgw80 = np.zeros((QKR, 2), f32)
    dgw80[0:10, 0] = inputs["dg1_W"][0:10, 0]
    dgw80[32:42, 0] = inputs["dg1_W"][10:20, 0]
    dgw80[64:84, 1] = inputs["dg2_W"][:, 0]
    consts["dgw80"] = dgw80.astype(bf16)
    masks = np.zeros((QKR, H), f32)
    for h in range(H):
        r = R_CH[h]
        masks[r:r + 5, h] = 1.0
    consts["masks"] = masks
    consts["dgb"] = np.array([[inputs["dg1_b"][0], inputs["dg2_b"][0]]], bf16)
    sel_lo = np.zeros((128, 8), bf16)
    sel_hi = np.zeros((128, 8), bf16)
    e5_lo = np.zeros((8, 128), bf16)
    e5_hi = np.zeros((8, 128), bf16)
    p_lo = np.zeros((128, E), bf16)
    p_hi = np.zeros((128, E), bf16)
    for k in range(4):
        sel_lo[32 * k + 5, k] = 1.0
        sel_hi[32 * k + 5, 4 + k] = 1.0
        for j in range(5):
            e5_lo[k, 32 * k + j] = 1.0
            e5_hi[4 + k, 32 * k + j] = 1.0
            p_lo[32 * k + j, :] = inputs["proj_W"][5 * k + j, :]
            p_hi[32 * k + j, :] = inputs["proj_W"][20 + 5 * k + j, :]
    consts.update(sel_lo=sel_lo, sel_hi=sel_hi, e5_lo=e5_lo, e5_hi=e5_hi,
                  p_lo=p_lo, p_hi=p_hi)
    consts["projb"] = np.ascontiguousarray(
        inputs["proj_b"].astype(f32).reshape(E, 1))
    consts["i128"] = np.eye(128, dtype=f32).astype(bf16)
    return xta, consts


def kernel(**inputs):
    from concourse.bass_utils import run_bass_kernel_spmd
    import ml_dtypes

    nc = _build_program()
    xta, consts = _host_arrays(inputs)
    fp8 = ml_dtypes.float8_e4m3fn
    dg1 = np.ascontiguousarray(inputs["dynamic_graph1"]).astype(fp8)
    dg2 = np.ascontiguousarray(inputs["dynamic_graph2"]).astype(fp8)
    in_maps = []
    for c in range(NCORES):
        sl = slice(c * B_LOC, (c + 1) * B_LOC)
        m = {"xta": xta[sl], "dg1": dg1[sl], "dg2": dg2[sl]}
        m.update(consts)
        in_maps.append(m)
    res = run_bass_kernel_spmd(nc, in_maps, list(range(NCORES)))
    outT = np.concatenate([res.results[c]["outT"] for c in range(NCORES)], 0)
    return np.ascontiguousarray(outT.transpose(0, 2, 1)).astype(np.float32)


# revision 32
# speedup vs baseline: 1.1844x; 1.0318x over previous
"""Trainium2 Bass kernel for nn_FAA_51367808860389 (FAN-attention w/ dynamic-graph bias).

Strategy: data-parallel over batch B=32 across 8 cores (4 batches/core).
Everything computed in transposed orientation energyT[k,q]:
  - FAN q/k projections via ONE matmul per batch (stationary [41,80] with
    pa duplicated so cos- and sin-source rows land on distinct partitions),
    compact 80-row layout, per-head channel rows contiguous -> no masking.
  - energyT[k,q] per (head, k-block): K=5 matmul from row slices.
  - bias via diagonal-matmul: eT block += dg-block^T @ diag(w-block); dg
    streamed from HBM in fp8e4m3 (half DMA), one DMA per (batch, head).
  - exp on Act engine writes att directly in fp8; out-matmul uses fp8
    DoubleRow (pairs of k-blocks contracted per pass) with an appended
    ones column for the softmax denominators.
  - normalize + head-concat + out-projection fused via prepacked matrices.
Output produced transposed [40, 512] per batch; host transposes back.
"""
import numpy as np

B, N, E, H, D = 32, 512, 40, 8, 5
NCORES = 8
B_LOC = B // NCORES
SCALE = 1.0 / float(np.float32(E) ** 0.5)

# qT/kT row layout (identical for both): cos ch0-9 at rows 0-9 | pad 10-31 |
# sin ch10-19 at rows 32-41 | pad 42-63 | g ch20-39 at rows 64-83.
# 32-aligned groups (Act/DVE/matmul partition bases must be multiples of 32).
QKR = 84
R_CH = {h: (5 * h if h < 2 else (22 + 5 * h if h < 4 else 44 + 5 * h))
        for h in range(H)}

_PROG_CACHE = {}


def _build_program():
    if "nc" in _PROG_CACHE:
        return _PROG_CACHE["nc"]
    import concourse.bass as bass
    import concourse.tile as tile
    from concourse import bacc, mybir

    F32 = mybir.dt.float32
    BF16 = mybir.dt.bfloat16
    FP8 = mybir.dt.float8e4
    AF = mybir.ActivationFunctionType
    OP = mybir.AluOpType
    DR = mybir.MatmulPerfMode.DoubleRow

    nc = bacc.Bacc(None)
    dp = nc.declare_dram_parameter
    xta_d = dp("xta", [B_LOC, 41, N], BF16, isOutput=False)
    dg1_d = dp("dg1", [B_LOC, 4, N, N], FP8, isOutput=False)
    dg2_d = dp("dg2", [B_LOC, 4, N, N], FP8, isOutput=False)
    wq_d = dp("wq", [41, QKR], BF16, isOutput=False)
    wk_d = dp("wk", [41, QKR], BF16, isOutput=False)
    wv_d = dp("wv", [41, 30], BF16, isOutput=False)     # v fan weights
    dgw80_d = dp("dgw80", [QKR, 2], BF16, isOutput=False)
    masks_d = dp("masks", [QKR, H], F32, isOutput=False)
    dgb_d = dp("dgb", [1, 2], BF16, isOutput=False)
    sel_lo_d = dp("sel_lo", [128, 8], BF16, isOutput=False)
    sel_hi_d = dp("sel_hi", [128, 8], BF16, isOutput=False)
    e5_lo_d = dp("e5_lo", [8, 128], BF16, isOutput=False)
    e5_hi_d = dp("e5_hi", [8, 128], BF16, isOutput=False)
    p_lo_d = dp("p_lo", [128, E], BF16, isOutput=False)
    p_hi_d = dp("p_hi", [128, E], BF16, isOutput=False)
    projb_d = dp("projb", [E, 1], F32, isOutput=False)
    i128_d = dp("i128", [128, 128], BF16, isOutput=False)
    out_d = dp("outT", [B_LOC, E, N], F32, isOutput=True)

    lp = nc.allow_low_precision(reason="bf16/fp8 datapath validated vs reference")
    lp.__enter__()
    with tile.TileContext(nc) as tc:
        with (
            tc.tile_pool(name="const", bufs=1) as cp,
            tc.tile_pool(name="work", bufs=2) as wp,
            tc.tile_pool(name="persist", bufs=B_LOC) as pp,
            tc.tile_pool(name="wcolp", bufs=8 * B_LOC) as wcp,
            tc.tile_pool(name="dgp", bufs=B_LOC) as dgpool,
            tc.tile_pool(name="kmp", bufs=B_LOC) as kmp,
            tc.tile_pool(name="attp", bufs=6) as attp,
            tc.tile_pool(name="psE", bufs=2, space=bass.MemorySpace.PSUM) as psE,
            tc.tile_pool(name="psO", bufs=1, space=bass.MemorySpace.PSUM) as psO,
            tc.tile_pool(name="psS", bufs=2, space=bass.MemorySpace.PSUM) as psS,
        ):
            # ---- constants to SBUF ----
            def cload(dram, shape, tag, dt=F32):
                t = cp.tile(shape, dt, tag=tag)
                nc.sync.dma_start(t[:], dram[:])
                return t

            wq = cload(wq_d, [41, QKR], "wq", BF16)
            wk = cload(wk_d, [41, QKR], "wk", BF16)
            wv = cload(wv_d, [41, 30], "wv", BF16)
            dgw80 = cload(dgw80_d, [QKR, 2], "dgw80", BF16)
            masks = cload(masks_d, [QKR, H], "masks")
            dgb = cload(dgb_d, [1, 2], "dgb", BF16)
            sel_lo = cload(sel_lo_d, [128, 8], "sel_lo", BF16)
            sel_hi = cload(sel_hi_d, [128, 8], "sel_hi", BF16)
            e5_lo = cload(e5_lo_d, [8, 128], "e5_lo", BF16)
            e5_hi = cload(e5_hi_d, [8, 128], "e5_hi", BF16)
            p_lo = cload(p_lo_d, [128, E], "p_lo", BF16)
            p_hi = cload(p_hi_d, [128, E], "p_hi", BF16)
            projb = cload(projb_d, [E, 1], "projb")
            i128 = cload(i128_d, [128, 128], "i128", BF16)
            ones_row = cp.tile([1, 128], BF16, tag="ones_row")
            nc.vector.memset(ones_row[:], 1.0)
            # PE warm-up: HAM throttles the PE to half clock until it sees
            # ~3us of continuous matmul activity. Burn a few back-to-back
            # dummy matmuls at the start (and between phases) so the whole
            # main loop runs at full clock.
            warm_src = cp.tile([128, N], BF16, tag="warm_src")
            nc.vector.memset(warm_src[:], 0.0)

            def pe_warm(k, pool=None):
                # HAM only un-throttles the PE on sustained full-array
                # matmul activity, so warms must be real M=128 matmuls.
                # Phase warms write the psO bank (dead until the first real
                # out-matmul, which has start=True); main-loop warms use the
                # psS fan bank (idle there). WAW chains keep them PE-only.
                p = pool or psO
                tag = "out_lo" if p is psO else "fan"
                for _ in range(k):
                    pw = p.tile([128, N], F32, tag=tag)
                    nc.tensor.matmul(pw[:], i128[:], warm_src[:],
                                     start=True, stop=True)

            pe_warm(8)

            # ---- xta loads + dg prefetch (all upfront) ----
            xta = []
            for b in range(B_LOC):
                xt = pp.tile([41, N], BF16, tag="xta")
                nc.sync.dma_start(xt[:], xta_d[b][:])
                xta.append(xt)
            # dg tiles: [128, 2048] fp8, free = r*512 + n (r = q-row-block)
            dgt = [[None] * H for _ in range(B_LOC)]
            for b in range(B_LOC):
                for h in range(H):
                    src = dg1_d if h < 4 else dg2_d
                    t = dgpool.tile([128, 4 * N], FP8, tag=f"dg{h}")
                    nc.sync.dma_start(
                        t[:].rearrange("p (r n) -> p r n", r=4),
                        src[b, h % 4].rearrange("(r p) n -> p r n", p=128))
                    dgt[b][h] = t

            # ---- phase F/V/W prep, batch-major ----
            # All trig-table Act ops (fan + v sins) are emitted before any
            # exp-table op; gates use sigmoid-via-exp so they live in the
            # exp table and can interleave with the main loop. DVE prep is
            # batch-major so batch b's masks/diag/vaug are ready ~in time
            # for its main loop.
            def fan_qk(b, w, tag):
                ps = psS.tile([QKR, N], F32, tag="fan")
                nc.tensor.matmul(ps[:], w[:], xta[b][:], start=True, stop=True)
                qk = pp.tile([QKR, N], BF16, tag=tag)
                s2 = wp.tile([64, N], BF16, tag="s2")
                s4 = wp.tile([64, N], BF16, tag="s4")
                # rows 0-9: pa for cos; rows 32-41: pa again for sin.
                # Ranges extend over the zero pad rows so no SBUF garbage
                # (NaN) survives into qk (0 * NaN = NaN). bf16 intermediates
                # enable the DVE 2x packed mode.
                nc.scalar.activation(s2[0:64, :], ps[0:64, :], AF.Sin, scale=0.5)
                nc.scalar.activation(s4[32:64, :], ps[32:64, :], AF.Sin,
                                     scale=0.25)
                sq2 = wp.tile([64, N], BF16, tag="sq2")
                nc.vector.tensor_tensor(sq2[0:32, :], s2[0:32, :], s2[0:32, :],
                                        op=OP.mult)
                # cos rows -> qk[0:10] (pad rows get 1.0, masked out later)
                nc.vector.tensor_scalar(qk[0:32, :], sq2[0:32, :], -2.0, 1.0,
                                        op0=OP.mult, op1=OP.add)
                # sin rows: c4 = 1-2*s4^2 ; sin = 2*s2*c4 -> qk[32:42]
                sq4 = wp.tile([64, N], BF16, tag="sq4")
                nc.vector.tensor_tensor(sq4[32:64, :], s4[32:64, :],
                                        s4[32:64, :], op=OP.mult)
                c4 = wp.tile([64, N], BF16, tag="c4")
                nc.vector.tensor_scalar(c4[32:64, :], sq4[32:64, :], -2.0, 1.0,
                                        op0=OP.mult, op1=OP.add)
                nc.vector.scalar_tensor_tensor(qk[32:64, :], s2[32:64, :], 2.0,
                                               c4[32:64, :], op0=OP.mult,
                                               op1=OP.mult)
                # g rows
                nc.vector.tensor_copy(qk[64:QKR, :], ps[64:QKR, :])
                return qk

            kTm = [None] * B_LOC

            def emit_masks(b):
                kTm[b] = []
                for h in range(H):
                    km = kmp.tile([QKR, N], BF16, tag=f"kTm{h}")
                    nc.vector.tensor_scalar(km[:], KT[b][:],
                                            masks[:, h:h + 1], None,
                                            op0=OP.mult)
                    kTm[b].append(km)

            def emit_v(b):
                va = pp.tile([128, 4 * 48], FP8, tag="vaug")
                nc.vector.memset(va[:], 1.0)
                for c in range(4):
                    ps = psS.tile([128, 30], F32, tag="fan")
                    nc.tensor.matmul(ps[:], xta[b][:, 128 * c:128 * (c + 1)],
                                     wv[:], start=True, stop=True)
                    s2v = wp.tile([128, 10], BF16, tag="s2v")
                    s4v = wp.tile([128, 10], BF16, tag="s4v")
                    nc.scalar.activation(s2v[:], ps[:, 0:10], AF.Sin, scale=0.5)
                    nc.scalar.activation(s4v[:], ps[:, 0:10], AF.Sin, scale=0.25)
                    sq2v = wp.tile([128, 10], BF16, tag="sq2v")
                    nc.vector.tensor_tensor(sq2v[:], s2v[:], s2v[:], op=OP.mult)
                    cosv = va[:, 48 * c:48 * c + 12].rearrange(
                        "p (h j) -> p h j", h=2)[:, :, 0:5]
                    nc.vector.tensor_scalar(
                        cosv, sq2v[:].rearrange("p (h j) -> p h j", h=2),
                        -2.0, 1.0, op0=OP.mult, op1=OP.add)
                    sq4v = wp.tile([128, 10], BF16, tag="sq4v")
                    c4v = wp.tile([128, 10], BF16, tag="c4v")
                    nc.vector.tensor_tensor(sq4v[:], s4v[:], s4v[:], op=OP.mult)
                    nc.vector.tensor_scalar(c4v[:], sq4v[:], -2.0, 1.0,
                                            op0=OP.mult, op1=OP.add)
                    sinv = wp.tile([128, 10], BF16, tag="sinv")
                    nc.vector.scalar_tensor_tensor(sinv[:], s2v[:], 2.0, c4v[:],
                                                   op0=OP.mult, op1=OP.mult)
                    sin_dst = va[:, 48 * c + 12:48 * c + 24].rearrange(
                        "p (h j) -> p h j", h=2)[:, :, 0:5]
                    nc.vector.tensor_copy(
                        sin_dst, sinv[:].rearrange("p (h j) -> p h j", h=2))
                    g_dst = va[:, 48 * c + 24:48 * c + 48].rearrange(
                        "p (h j) -> p h j", h=4)[:, :, 0:5]
                    nc.vector.tensor_copy(
                        g_dst, ps[:, 10:30].rearrange("p (h j) -> p h j", h=4))
                return va

            dgws = [[[None] * 4, [None] * 4] for _ in range(B_LOC)]

            def emit_gates(b):
                # w = sigmoid(z) = 1 / (1 + exp(-z)): Exp is in the same act
                # table as the main-loop exps, so no table reloads.
                for wsel in range(2):
                    for blk in range(4):
                        sl = slice(128 * blk, 128 * (blk + 1))
                        zp = psS.tile([128, 1], F32, tag="fan")
                        nc.tensor.matmul(zp[:], QT[b][:, sl],
                                         dgw80[:, wsel:wsel + 1],
                                         start=True, stop=False)
                        nc.tensor.matmul(zp[:], ones_row[:],
                                         dgb[:, wsel:wsel + 1],
                                         start=False, stop=True)
                        ge = wcp.tile([128, 1], F32, tag="th")
                        nc.scalar.activation(ge[:], zp[:], AF.Exp, scale=-1.0)
                        e1 = wcp.tile([128, 1], F32, tag="e1")
                        nc.vector.tensor_scalar(e1[:], ge[:], 1.0, None,
                                                op0=OP.add)
                        wc = wcp.tile([128, 1], F32, tag="wcol")
                        nc.vector.reciprocal_approx_fast(wc[:], e1[:])
                        dw = wcp.tile([128, 128], BF16, tag="dgw")
                        nc.vector.tensor_scalar(dw[:], i128[:], wc[:], None,
                                                op0=OP.mult)
                        dgws[b][wsel][blk] = dw

            QT, KT = [None] * B_LOC, [None] * B_LOC
            vaug = [None] * B_LOC
            for b in range(B_LOC):
                QT[b] = fan_qk(b, wq, "QT")
                KT[b] = fan_qk(b, wk, "KT")
                emit_masks(b)
                vaug[b] = emit_v(b)

            # PE filler while the Act/DVE queues drain the prep ahead of the
            # first exps; executes during the wait, keeps HAM at K=8
            pe_warm(40)
            emit_gates(0)

            # ---- main loop ----
            # Software-pipelined: out-matmuls trail the energy/bias stream
            # by LAG pair-groups so the PE never stalls on an exp. Gates of
            # b+1 and stage5 of b-1 are folded into batch b's stream.
            LAG = 2

            def emit_stage5(b, out_lo, out_hi):
                sb_lo = wp.tile([128, N], BF16, tag="sb_lo")
                sb_hi = wp.tile([128, N], BF16, tag="sb_hi")
                nc.vector.tensor_copy(sb_lo[:], out_lo[:])
                nc.vector.tensor_copy(sb_hi[:], out_hi[:])
                sums8 = psS.tile([128, N], F32, tag="fan")
                nc.tensor.matmul(sums8[0:8, :], sel_lo[:], sb_lo[:],
                                 start=True, stop=False)
                nc.tensor.matmul(sums8[0:8, :], sel_hi[:], sb_hi[:],
                                 start=False, stop=True)
                recipf = wp.tile([8, N], F32, tag="recipf")
                nc.vector.reciprocal_approx_fast(recipf[:], sums8[0:8, :])
                recip8 = wp.tile([8, N], BF16, tag="recip8")
                nc.vector.tensor_copy(recip8[:], recipf[:])
                rm_lo = psS.tile([128, N], F32, tag="fan")
                nc.tensor.matmul(rm_lo[:], e5_lo[:], recip8[:],
                                 start=True, stop=True)
                sbn_lo = wp.tile([128, N], BF16, tag="sbn_lo")
                nc.vector.tensor_tensor(sbn_lo[:], sb_lo[:], rm_lo[:],
                                        op=OP.mult)
                rm_hi = psS.tile([128, N], F32, tag="fan")
                nc.tensor.matmul(rm_hi[:], e5_hi[:], recip8[:],
                                 start=True, stop=True)
                sbn_hi = wp.tile([128, N], BF16, tag="sbn_hi")
                nc.vector.tensor_tensor(sbn_hi[:], sb_hi[:], rm_hi[:],
                                        op=OP.mult)
                prj = psS.tile([128, N], F32, tag="fan")
                nc.tensor.matmul(prj[0:E, :], p_lo[:], sbn_lo[:],
                                 start=True, stop=False)
                nc.tensor.matmul(prj[0:E, :], p_hi[:], sbn_hi[:],
                                 start=False, stop=True)
                out_sb = wp.tile([E, N], F32, tag="out_sb")
                nc.scalar.activation(out_sb[:], prj[0:E, :], AF.Identity,
                                     bias=projb[:])
                nc.sync.dma_start(out_d[b][:], out_sb[:])

            def pop_out(entry):
                ops_, ob_, b_, h_, pair_, att2_ = entry
                for half in range(2):
                    j_ = 2 * pair_ + half
                    nc.tensor.matmul(
                        ops_[ob_:ob_ + 6, :],
                        vaug[b_][:, 48 * j_ + 6 * h_:48 * j_ + 6 * h_ + 6],
                        att2_[:, N * half:N * (half + 1)],
                        start=(j_ == 0), stop=(j_ == 3),
                        tile_position=(0, ob_), skip_group_check=True)

            outq = []
            prev_out = None
            for b in range(B_LOC):
                out_lo = psO.tile([128, N], F32, tag="out_lo")
                out_hi = psO.tile([128, N], F32, tag="out_hi")
                for h in range(H):
                    wsel = 0 if h < 4 else 1
                    out_ps = out_lo if h < 4 else out_hi
                    obase = 32 * (h % 4)
                    for pair in range(2):
                        eT2 = psE.tile([128, 2 * N], F32, tag="eT")
                        for half in range(2):
                            j = 2 * pair + half
                            sl = slice(N * half, N * (half + 1))
                            nc.tensor.matmul(
                                eT2[:, sl],
                                kTm[b][h][:, 128 * j:128 * (j + 1)],
                                QT[b][:], start=True, stop=False,
                                skip_group_check=True)
                            for i in range(4):
                                nc.tensor.matmul(
                                    eT2[:, N * half + 128 * i:
                                        N * half + 128 * (i + 1)],
                                    dgt[b][h][:, 512 * i + 128 * j:
                                              512 * i + 128 * j + 128],
                                    dgws[b][wsel][i][:],
                                    start=False, stop=(i == 3),
                                    skip_group_check=True)
                        att2 = attp.tile([128, 2 * N], FP8, tag="att2")
                        nc.scalar.activation(att2[:], eT2[:], AF.Exp,
                                             scale=SCALE)
                        outq.append((out_ps, obase, b, h, pair, att2))
                        while len(outq) > LAG:
                            pop_out(outq.pop(0))
                    if h == 1 and b + 1 < B_LOC:
                        emit_gates(b + 1)
                    if h == 2 and prev_out is not None:
                        emit_stage5(b - 1, prev_out[0], prev_out[1])
                        prev_out = None
                prev_out = (out_lo, out_hi)
            while outq:
                pop_out(outq.pop(0))
            emit_stage5(B_LOC - 1, prev_out[0], prev_out[1])

    lp.__exit__(None, None, None)
    nc.compile()
    _PROG_CACHE["nc"] = nc
    return nc


def _host_arrays(inputs):
    import ml_dtypes
    bf16 = ml_dtypes.bfloat16
    f32 = np.float32
    x = np.ascontiguousarray(inputs["x"], dtype=f32)
    ones = np.ones((B, 1, N), f32)
    xta = np.ascontiguousarray(
        np.concatenate([x.transpose(0, 2, 1), ones], axis=1)).astype(bf16)

    def aug(wp_, bp_):
        return np.concatenate([wp_, bp_[None, :]], 0).astype(f32)

    consts = {}
    qp = aug(inputs["q_Wp"], inputs["q_bp"])
    kp = aug(inputs["k_Wp"], inputs["k_bp"])
    qg = aug(inputs["q_Wg"], inputs["q_bg"])
    kg = aug(inputs["k_Wg"], inputs["k_bg"])
    # cols per proj: pa(cos) 0-9 | pad 10-31 | pa(sin) 32-41 | pad 42-63 |
    # g 64-83  (pads keep 32-aligned partition groups)
    z22 = np.zeros((41, 22), f32)
    consts["wq"] = np.ascontiguousarray(
        np.concatenate([qp, z22, qp, z22, qg], axis=1)).astype(bf16)
    consts["wk"] = np.ascontiguousarray(
        np.concatenate([kp, z22, kp, z22, kg], axis=1)).astype(bf16)
    vp = aug(inputs["v_Wp"], inputs["v_bp"])
    vg = aug(inputs["v_Wg"], inputs["v_bg"])
    consts["wv"] = np.ascontiguousarray(
        np.concatenate([vp, vg], axis=1)).astype(bf16)
    # gate vectors padded to the QK row layout
    dgw80 = np.zeros((QKR, 2), f32)
    dgw80[0:10, 0] = inputs["dg1_W"][0:10, 0]
    dgw80[32:42, 0] = inputs["dg1_W"][10:20, 0]
    dgw80[64:84, 1] = inputs["dg2_W"][:, 0]
    consts["dgw80"] = dgw80.astype(bf16)
    masks = np.zeros((QKR, H), f32)
    for h in range(H):
        r = R_CH[h]
        masks[r:r + 5, h] = 1.0
    consts["masks"] = masks
    consts["dgb"] = np.array([[inputs["dg1_b"][0], inputs["dg2_b"][0]]], bf16)
    sel_lo = np.zeros((128, 8), bf16)
    sel_hi = np.zeros((128, 8), bf16)
    e5_lo = np.zeros((8, 128), bf16)
    e5_hi = np.zeros((8, 128), bf16)
    p_lo = np.zeros((128, E), bf16)
    p_hi = np.zeros((128, E), bf16)
    for k in range(4):
        sel_lo[32 * k + 5, k] = 1.0
        sel_hi[32 * k + 5, 4 + k] = 1.0
        for j in range(5):
            e5_lo[k, 32 * k + j] = 1.0
            e5_hi[4 + k, 32 * k + j] = 1.0
            p_lo[32 * k + j, :] = inputs["proj_W"][5 * k + j, :]
            p_hi[32 * k + j, :] = inputs["proj_W"][20 + 5 * k + j, :]
    consts.update(sel_lo=sel_lo, sel_hi=sel_hi, e5_lo=e5_lo, e5_hi=e5_hi,
                  p_lo=p_lo, p_hi=p_hi)
    consts["projb"] = np.ascontiguousarray(
        inputs["proj_b"].astype(f32).reshape(E, 1))
    consts["i128"] = np.eye(128, dtype=f32).astype(bf16)
    return xta, consts


def kernel(**inputs):
    from concourse.bass_utils import run_bass_kernel_spmd
    import ml_dtypes

    nc = _build_program()
    xta, consts = _host_arrays(inputs)
    fp8 = ml_dtypes.float8_e4m3fn
    dg1 = np.ascontiguousarray(inputs["dynamic_graph1"]).astype(fp8)
    dg2 = np.ascontiguousarray(inputs["dynamic_graph2"]).astype(fp8)
    in_maps = []
    for c in range(NCORES):
        sl = slice(c * B_LOC, (c + 1) * B_LOC)
        m = {"xta": xta[sl], "dg1": dg1[sl], "dg2": dg2[sl]}
        m.update(consts)
        in_maps.append(m)
    res = run_bass_kernel_spmd(nc, in_maps, list(range(NCORES)))
    outT = np.concatenate([res.results[c]["outT"] for c in range(NCORES)], 0)
    return np.ascontiguousarray(outT.transpose(0, 2, 1)).astype(np.float32)
